# revision 20
# baseline (speedup 1.0000x reference)
"""Multi-head causal self-attention on 8 Trainium2 NeuronCores.

Sharding: tensor-parallel over heads -- 16 heads / 8 cores = 2 heads per
core.  Every core receives the full activations x (replicated, bf16) plus
the W_Q/W_K/W_V/W_O slices for its 2 heads, computes attention + output
projection for those heads, and writes a partial (B,S,D) bf16 output.
The all-reduce over heads is done on the host by summing the 8 partials
(b_O and the exact b_V fold are also added on the host).

Device algorithm per core (heads h0, h1), per batch b:
  - x^T (D,S) bf16 staged in SBUF, DMA'd in s-major halves so the first
    projections can start after ~half the batch has landed.
  - Q^T,K^T (128=2*DH, S) projections with both heads stacked on the
    output-partition axis.  V is projected directly into natural
    (Sk, 2*DH) layout (x^T chunk as the stationary operand), so no PE
    transposes are needed; ones columns per head are memset for the
    softmax denominators.
  - scores^T (Sk,Sq) = K^T.T @ Q^T per 128x512 block, both heads packed
    into one concurrent PE pass (K=64 row groups 0-1 / 2-3).  Fully
    masked blocks are skipped; diagonal blocks are column-trimmed and
    both heads exp'd in ONE ScalarE activation; the 0/1 causal triangle
    multiply only touches the 128-wide diagonal strip.
  - z^T (65,Sq) = V_aug.T @ expS accumulated over Sk; row 64 = softmax
    denominators (ones column).
  - normalize: denominator rows are DMA-reshaped across 128 partitions,
    reciprocal'd there, DMA'd back as a (2,Sq) tile; ONE K=2 matmul
    against a 0/1 selector broadcasts both heads' 1/s to the matching
    partition halves; z (re-staged as a (128,Sq) tile, head1 lane-
    shifted by DMA off the critical path) is normalized by a single DVE
    multiply feeding the output projection (K=128, one matmul per tile).
  - the whole thing is software-pipelined ACROSS rounds: round r's PE
    stream is [scores(r) | z(r-1) interleaved | bcast+oproj(r-2)], so
    the PE never waits on the exp or the reciprocal/normalize chain and
    the PE HAM clock-gate stays at 2.4 GHz.  ~6 us of warmup matmuls at
    t0 cover the initial x DMA so the real stream starts warm.

Matmul operands are bf16 (fp32 PSUM accumulation) except the reciprocal
broadcast which runs in float32r.  Output partials are written bf16 and
summed on the host in fp32.
"""

import sys

import numpy as np

sys.path.insert(0, "/opt/trn_rl_repo")

# Problem dims (hardcoded per contract -- kernel.py must be self-contained).
B, S, D, H, DH = 4, 2048, 1024, 16, 64
N_CORES = 8
HPC = H // N_CORES  # heads per core = 2
SCALE = 1.0 / float(np.sqrt(DH))

NQ = 512  # q-chunk width (PSUM bank)
KT = 128  # k-tile height (partitions)
NWARM = 14  # HAM warmup matmuls at t0


def build_program(b_dim=B, s_dim=S, d_dim=D, num_devices=N_CORES):
    """Build the per-core Bass program (same program on every core)."""
    from concourse import bacc, mybir, tile

    f32 = mybir.dt.float32
    f32r = mybir.dt.float32r
    bf16 = mybir.dt.bfloat16
    act = mybir.ActivationFunctionType

    KC = d_dim // 128  # contraction chunks for projections
    SQC = s_dim // NQ  # q chunks per batch
    RPQ = NQ // KT  # k tiles per q chunk on the diagonal (4)
    xw = NQ  # x DMA granularity (one q-chunk -> projections start early)

    nc = bacc.Bacc(
        "TRN2",
        target_bir_lowering=False,
        debug=False,
        enable_asserts=False,
        num_devices=num_devices,
    )

    xT = nc.dram_tensor("xT", [b_dim, d_dim, s_dim], bf16, kind="ExternalInput").ap()
    wq_d = nc.dram_tensor("wq", [128, KC, 128], bf16, kind="ExternalInput").ap()
    wk_d = nc.dram_tensor("wk", [128, KC, 128], bf16, kind="ExternalInput").ap()
    wv_d = nc.dram_tensor("wv", [128, KC, 128], bf16, kind="ExternalInput").ap()
    wo_d = nc.dram_tensor("wo", [128, d_dim], bf16, kind="ExternalInput").ap()
    bq_d = nc.dram_tensor("bq", [128, 1], f32, kind="ExternalInput").ap()
    bk_d = nc.dram_tensor("bk", [128, 1], f32, kind="ExternalInput").ap()
    tri_d = nc.dram_tensor("tri", [128, 128], bf16, kind="ExternalInput").ap()
    sel_d = nc.dram_tensor("sel", [2, 128], f32, kind="ExternalInput").ap()
    out_d = nc.dram_tensor("out", [b_dim, s_dim, d_dim], bf16, kind="ExternalOutput").ap()

    with tile.TileContext(nc) as tc:
        with (
            tc.tile_pool(name="singles", bufs=1) as singles,
            tc.tile_pool(name="xpool", bufs=2 * KC * (s_dim // NQ)) as xpool,
            tc.tile_pool(name="qkpool", bufs=2) as qkpool,
            tc.tile_pool(name="vpool", bufs=2 * (s_dim // KT) + 2) as vpool,
            tc.tile_pool(name="epool", bufs=20) as epool,
            tc.tile_pool(name="fpool", bufs=2) as fpool,
            tc.tile_pool(name="opool", bufs=3) as opool,
            tc.tile_pool(name="ps_s", bufs=2, space="PSUM") as ps_s,
            tc.tile_pool(name="ps_z", bufs=2, space="PSUM") as ps_z,
            tc.tile_pool(name="ps_m", bufs=2, space="PSUM") as ps_m,
        ):
            # ---- warmup: keep PE busy (HAM warm) while batch-0 x lands ----
            warm_sb = singles.tile([128, NQ], bf16)
            nc.gpsimd.memset(warm_sb, 0.0)
            for i in range(NWARM):
                pw = ps_m.tile([128, NQ], f32, name=f"warm_{i}", tag="m")
                nc.tensor.matmul(
                    pw, lhsT=warm_sb[:, 0:128], rhs=warm_sb, start=True, stop=True
                )

            # ---- constants / weights (loaded once) ----
            wq_sb = singles.tile([128, KC, 128], bf16)
            wk_sb = singles.tile([128, KC, 128], bf16)
            wv_sb = singles.tile([128, KC, 128], bf16)
            wo_sb = singles.tile([128, d_dim], bf16)
            bq_sb = singles.tile([128, 1], f32)
            bk_sb = singles.tile([128, 1], f32)
            tri_sb = singles.tile([128, 128], bf16)

            nc.sync.dma_start(out=wq_sb, in_=wq_d)
            nc.sync.dma_start(out=wk_sb, in_=wk_d)
            nc.sync.dma_start(out=wv_sb, in_=wv_d)
            nc.sync.dma_start(out=wo_sb, in_=wo_d)
            nc.sync.dma_start(out=bq_sb, in_=bq_d)
            nc.sync.dma_start(out=bk_sb, in_=bk_d)
            nc.sync.dma_start(out=tri_sb, in_=tri_d)

            # 0/1 selector for the merged reciprocal broadcast:
            # pr = sel2.T @ rrow2 puts head0's 1/s on partitions 0-63 and
            # head1's on 64-127 in one K=2 matmul.
            sel2f = singles.tile([2, 128], f32)
            nc.sync.dma_start(out=sel2f, in_=sel_d)
            sel2 = singles.tile([2, 128], f32r)
            nc.vector.tensor_copy(sel2, sel2f)

            # ---------------- pipelined rounds ----------------
            # round r state: (b, qc, eps, v_tiles, pz0, pz1) then after fp1:
            # (zf, rrow2); fp2 consumes those two rounds later.
            # qc order [1,2,3,0]: the kernel drains on a 4-tile round, not 16
            rounds = [(b, qc) for b in range(b_dim) for qc in (1, 2, 3, 0)[:SQC]]

            st = [dict() for _ in rounds]  # per-round tiles
            v_tiles_b = {}  # batch -> list of 16 V tiles

            def emit_fp1(i):
                """Normalize prep for round i: restage z + reciprocal chain.
                Emitted right after round i's z matmuls are all in."""
                b_p, qc_p = rounds[i]
                s = st[i]
                pz0, pz1 = s["pz0"], s["pz1"]
                zf = fpool.tile([128, NQ], f32, name=f"zf_{i}", tag="zf")
                ztmp = fpool.tile([65, 2 * NQ], f32, name=f"zt_{i}", tag="zt")
                # head0 e-rows straight in; head1 e-rows + denom via ztmp
                nc.vector.tensor_copy(zf[0:DH, :], pz0[0:DH, :])
                nc.vector.tensor_copy(ztmp[0:DH + 1, 0:NQ], pz1)
                nc.vector.tensor_copy(
                    ztmp[DH : DH + 1, NQ : 2 * NQ], pz0[DH : DH + 1, :]
                )
                # lane-shift head1 e-rows to partitions 64-127 (DMA crosses
                # lanes; compute engines can't) -- off the critical path.
                nc.sync.dma_start(out=zf[DH:128, :], in_=ztmp[0:DH, 0:NQ])
                # reciprocal of both denominator rows, spread over 128 lanes
                rs = fpool.tile([128, 8], f32, name=f"rs_{i}", tag="rs")
                rr8 = fpool.tile([128, 8], f32r, name=f"rr8_{i}", tag="rr8")
                rrow2 = fpool.tile([2, NQ], f32r, name=f"rw_{i}", tag="rw")
                nc.sync.dma_start(out=rs[:, 0:4], in_=ztmp[DH : DH + 1, NQ : 2 * NQ])
                nc.sync.dma_start(out=rs[:, 4:8], in_=ztmp[DH : DH + 1, 0:NQ])
                with nc.allow_low_precision("fp32r feed for PE broadcast"):
                    nc.vector.reciprocal(rr8, rs)
                nc.sync.dma_start(out=rrow2[0:1, :], in_=rr8[:, 0:4])
                nc.sync.dma_start(out=rrow2[1:2, :], in_=rr8[:, 4:8])
                s["zf"] = zf
                s["rrow2"] = rrow2

            def emit_fp2a(i):
                """Broadcast 1/s (one K=2 matmul) + normalize multiply."""
                s = st[i]
                pr = ps_m.tile([128, NQ], f32, name=f"pr_{i}", tag="m")
                nc.tensor.matmul(pr, lhsT=sel2, rhs=s["rrow2"], start=True, stop=True)
                znS = fpool.tile([128, NQ], bf16, name=f"zn_{i}", tag="zn")
                nc.vector.tensor_mul(znS, s["zf"], pr)
                s["znS"] = znS

            def emit_fp2b(i):
                """Output projection of round i + store (bf16)."""
                b_p, qc_p = rounds[i]
                znS = st[i]["znS"]
                for mt in range(NQ // 128):
                    ob = opool.tile([128, d_dim], bf16, name=f"ob_{i}_{mt}", tag="ob")
                    msl = slice(mt * 128, (mt + 1) * 128)
                    for n2 in range((d_dim + NQ - 1) // NQ):
                        nw = min(NQ, d_dim - n2 * NQ)
                        nsl = slice(n2 * NQ, n2 * NQ + nw)
                        po = ps_m.tile([128, nw], f32, name=f"po_{i}_{mt}_{n2}", tag="m")
                        nc.tensor.matmul(
                            po, lhsT=znS[:, msl], rhs=wo_sb[:, nsl], start=True, stop=True
                        )
                        nc.vector.tensor_copy(ob[:, nsl], po)
                    nc.sync.dma_start(
                        out=out_d[b_p, qc_p * NQ + mt * 128 : qc_p * NQ + (mt + 1) * 128, :],
                        in_=ob,
                    )

            for i, (b, qc) in enumerate(rounds):
                nkt_q = RPQ * qc + RPQ

                # ---- z-interleave state for the previous round ----
                zkt = 0
                nkt_p = 0
                if i > 0:
                    b_p, qc_p = rounds[i - 1]
                    nkt_p = RPQ * qc_p + RPQ
                    eps_p = st[i - 1]["eps"]
                    v_p = v_tiles_b[b_p]
                    pz0 = ps_z.tile([DH + 1, NQ], f32, name=f"pz0_{i-1}", tag="z")
                    pz1 = ps_z.tile([DH + 1, NQ], f32, name=f"pz1_{i-1}", tag="z")
                    st[i - 1]["pz0"] = pz0
                    st[i - 1]["pz1"] = pz1

                    def emit_z_pair():
                        nonlocal zkt
                        vsb = v_p[zkt]
                        ep_p, zq0 = eps_p[zkt]
                        nc.tensor.matmul(
                            pz0[:, zq0:NQ],
                            lhsT=vsb[:, 0 : DH + 1],
                            rhs=ep_p[:, zq0:NQ],
                            start=(zkt == 0),
                            stop=(zkt == nkt_p - 1),
                        )
                        nc.tensor.matmul(
                            pz1[:, zq0:NQ],
                            lhsT=vsb[:, DH + 1 : 2 * DH + 2],
                            rhs=ep_p[:, NQ + zq0 : 2 * NQ],
                            start=(zkt == 0),
                            stop=(zkt == nkt_p - 1),
                        )
                        zkt += 1

                def pace_z(frac):
                    # emit z(prev) pairs front-loaded (x1.45) so the last pair
                    # lands ~70% through this round -- the reciprocal chain then
                    # completes before the next round's broadcast matmul needs it
                    if i > 0:
                        target = int(frac * 1.45 * nkt_p + 1e-9)
                        while zkt < min(nkt_p, target):
                            emit_z_pair()

                fp2_i = i - 2  # round whose finalize part 2 runs now

                if i % SQC == 0:
                    # ---- batch start: x DMA + projections ----
                    xk = [[None] * (s_dim // NQ) for _ in range(KC)]
                    for q8 in range(s_dim // xw):
                        for k in range(KC):
                            xt = xpool.tile([128, xw], bf16, name=f"x_{b}_{k}_{q8}", tag="x")
                            nc.sync.dma_start(
                                out=xt,
                                in_=xT[b, k * 128 : (k + 1) * 128, q8 * xw : (q8 + 1) * xw],
                            )
                            for j in range(xw // NQ):
                                xk[k][q8 * (xw // NQ) + j] = xt[:, j * NQ : (j + 1) * NQ]

                    QT = qkpool.tile([128, s_dim], bf16, name=f"QT_{b}", tag="QT")
                    KTt = qkpool.tile([128, s_dim], bf16, name=f"KT_{b}", tag="KT")
                    v_tiles = []
                    v_tiles_b[b] = v_tiles
                    for q8 in range(2):
                        for dst, wsb, bias in ((QT, wq_sb, bq_sb), (KTt, wk_sb, bk_sb)):
                            for q4 in (2 * q8, 2 * q8 + 1):
                                sl = slice(q4 * NQ, (q4 + 1) * NQ)
                                pp = ps_m.tile(
                                    [128, NQ], f32, name=f"pp_{b}_{q4}_{dst.name}", tag="m"
                                )
                                for k in range(KC):
                                    nc.tensor.matmul(
                                        pp,
                                        lhsT=wsb[:, k, :],
                                        rhs=xk[k][q4],
                                        start=(k == 0),
                                        stop=(k == KC - 1),
                                    )
                                nc.vector.tensor_scalar_add(dst[:, sl], pp, bias)
                        # V in natural (seq, head-dim) layout: x^T chunk is the
                        # stationary operand, so no transpose is ever needed.
                        for kt in range(8 * q8, 8 * q8 + 8):
                            q4, j = kt // RPQ, kt % RPQ
                            pv = ps_m.tile([128, 128], f32, name=f"pv_{b}_{kt}", tag="m")
                            for k in range(KC):
                                nc.tensor.matmul(
                                    pv,
                                    lhsT=xk[k][q4][:, j * 128 : (j + 1) * 128],
                                    rhs=wv_sb[:, k, :],
                                    start=(k == 0),
                                    stop=(k == KC - 1),
                                )
                            vsb = vpool.tile(
                                [128, 2 * DH + 2], bf16, name=f"v_{b}_{kt}", tag="v"
                            )
                            nc.vector.tensor_copy(vsb[:, 0:DH], pv[:, 0:DH])
                            nc.vector.tensor_copy(
                                vsb[:, DH + 1 : 2 * DH + 1], pv[:, DH : 2 * DH]
                            )
                            nc.vector.memset(vsb[:, DH : DH + 1], 1.0)
                            nc.vector.memset(vsb[:, 2 * DH + 1 : 2 * DH + 2], 1.0)
                            v_tiles.append(vsb)
                        if q8 == 0:
                            pace_z(0.25)
                            if fp2_i >= 0:
                                emit_fp2a(fp2_i)
                        else:
                            pace_z(0.5)
                            if fp2_i >= 0:
                                emit_fp2b(fp2_i)
                    fp2a_done = fp2b_done = True
                else:
                    # broadcast matmul first: its reciprocal chain completed
                    # during the previous round (z is paced to finish early)
                    fp2a_done = fp2b_done = fp2_i < 0
                    if fp2_i >= 0:
                        emit_fp2a(fp2_i)
                        fp2a_done = True

                # ---- scores + exp (+ z(prev) interleave) ----
                qsl0 = qc * NQ
                eps_cur = []
                for kt in range(nkt_q):
                    ksl = slice(kt * KT, (kt + 1) * KT)
                    r = kt - RPQ * qc
                    q0 = 0 if r < 0 else KT * r  # valid columns start
                    sp = ps_s.tile([128, 2 * NQ], f32, name=f"sp_{i}_{kt}", tag="s")
                    nc.tensor.matmul(
                        sp[:, q0:NQ],
                        lhsT=KTt[0:DH, ksl],
                        rhs=QT[0:DH, qsl0 + q0 : qsl0 + NQ],
                        start=True,
                        stop=True,
                    )
                    nc.tensor.matmul(
                        sp[:, NQ + q0 : 2 * NQ],
                        lhsT=KTt[DH:128, ksl],
                        rhs=QT[DH:128, qsl0 + q0 : qsl0 + NQ],
                        start=True,
                        stop=True,
                    )
                    ep = epool.tile([128, 2 * NQ], bf16, name=f"ep_{i}_{kt}", tag="e")
                    if r < 0:
                        nc.scalar.activation(ep, sp, act.Exp, scale=SCALE)
                    else:
                        # one activation covers both heads (incl. the dead
                        # middle zone, which is never read)
                        nc.scalar.activation(
                            ep[:, q0 : 2 * NQ], sp[:, q0 : 2 * NQ], act.Exp, scale=SCALE
                        )
                        # causal 0/1 multiply only on the 128-wide diagonal strip
                        nc.vector.tensor_mul(
                            ep[:, q0 : q0 + KT], ep[:, q0 : q0 + KT], tri_sb
                        )
                        nc.vector.tensor_mul(
                            ep[:, NQ + q0 : NQ + q0 + KT],
                            ep[:, NQ + q0 : NQ + q0 + KT],
                            tri_sb,
                        )
                    eps_cur.append((ep, q0))

                    if not fp2b_done and kt == 2:
                        emit_fp2b(fp2_i)
                        fp2b_done = True
                    # pace every other kt: larger z blocks keep the PE's
                    # background weight loads pipelined instead of exposing
                    # one LDWEIGHTS per phase switch
                    if kt % 2 == 1 or kt == nkt_q - 1:
                        pace_z((kt + 1) / nkt_q)

                if not fp2b_done:
                    emit_fp2b(fp2_i)
                pace_z(1.0)
                if i > 0:
                    emit_fp1(i - 1)
                st[i]["eps"] = eps_cur

            # ---- drain the last round ----
            i = len(rounds) - 1
            b_p, qc_p = rounds[i]
            nkt_p = RPQ * qc_p + RPQ
            eps_p = st[i]["eps"]
            v_p = v_tiles_b[b_p]
            pz0 = ps_z.tile([DH + 1, NQ], f32, name=f"pz0_{i}", tag="z")
            pz1 = ps_z.tile([DH + 1, NQ], f32, name=f"pz1_{i}", tag="z")
            st[i]["pz0"] = pz0
            st[i]["pz1"] = pz1
            for zkt in range(nkt_p):
                vsb = v_p[zkt]
                ep_p, zq0 = eps_p[zkt]
                nc.tensor.matmul(
                    pz0[:, zq0:NQ],
                    lhsT=vsb[:, 0 : DH + 1],
                    rhs=ep_p[:, zq0:NQ],
                    start=(zkt == 0),
                    stop=(zkt == nkt_p - 1),
                )
                nc.tensor.matmul(
                    pz1[:, zq0:NQ],
                    lhsT=vsb[:, DH + 1 : 2 * DH + 2],
                    rhs=ep_p[:, NQ + zq0 : 2 * NQ],
                    start=(zkt == 0),
                    stop=(zkt == nkt_p - 1),
                )
                if zkt == min(1, nkt_p - 2):
                    emit_fp2a(i - 1)
            emit_fp2b(i - 1)
            emit_fp1(i)
            emit_fp2a(i)
            emit_fp2b(i)

    nc.compile()
    return nc


def to_bf16(a):
    import ml_dtypes

    return np.ascontiguousarray(np.asarray(a, dtype=np.float32)).astype(
        ml_dtypes.bfloat16
    )


def make_core_inputs(x, W_Q, b_Q, W_K, b_K, W_V, b_V, W_O, b_O):
    """Host-side prep: transpose x, slice + re-layout per-core weights."""
    b_dim, s_dim, d_dim = x.shape
    KC = d_dim // 128

    xT = to_bf16(np.transpose(x, (0, 2, 1)))  # (B, D, S)

    # causal 0/1 triangle for the 128-wide diagonal strips (valid iff q >= k)
    k_idx = np.arange(KT)[:, None]
    q_idx = np.arange(KT)[None, :]
    tri = to_bf16((q_idx >= k_idx).astype(np.float32))  # (128, 128)

    # 0/1 selector rows for the merged reciprocal broadcast matmul
    sel = np.zeros((2, 128), dtype=np.float32)
    sel[0, 0:DH] = 1.0
    sel[1, DH:128] = 1.0

    in_maps = []
    for c in range(N_CORES):
        h0, h1 = HPC * c, HPC * c + 1

        def stack2(w):  # (2 heads of (D, DH)) -> (128, KC, 128) chunked layout
            w2 = np.concatenate([w[h0], w[h1]], axis=1)  # (D, 128)
            return to_bf16(w2.reshape(KC, 128, 2 * DH).transpose(1, 0, 2))

        in_maps.append(
            {
                "xT": xT,
                "wq": stack2(W_Q),
                "wk": stack2(W_K),
                "wv": stack2(W_V),
                "wo": to_bf16(np.concatenate([W_O[h0], W_O[h1]], axis=0)),
                "bq": np.concatenate([b_Q[h0], b_Q[h1]]).reshape(128, 1).copy(),
                "bk": np.concatenate([b_K[h0], b_K[h1]]).reshape(128, 1).copy(),
                "tri": tri,
                "sel": sel,
            }
        )
    return in_maps


_PROGRAM_CACHE = {}


def run_cores(in_maps, trace=False, b_dim=B, s_dim=S, d_dim=D):
    from concourse import bass_utils

    key = (b_dim, s_dim, d_dim)
    if key not in _PROGRAM_CACHE:
        _PROGRAM_CACHE[key] = build_program(b_dim, s_dim, d_dim)
    nc = _PROGRAM_CACHE[key]
    res = bass_utils.run_bass_kernel_spmd(
        nc, in_maps, core_ids=list(range(len(in_maps))), trace=trace
    )
    return res


def kernel(x, W_Q, b_Q, W_K, b_K, W_V, b_V, W_O, b_O, _trace=False, _results=None):
    x = np.asarray(x, dtype=np.float32)
    in_maps = make_core_inputs(x, W_Q, b_Q, W_K, b_K, W_V, b_V, W_O, b_O)
    res = run_cores(in_maps, trace=_trace)
    if _results is not None:
        _results.append(res)
    out = np.zeros((B, S, D), dtype=np.float32)
    for r in res.results:
        out += np.asarray(r["out"], dtype=np.float32)
    # bias folds done on host: b_O directly; b_V's exact effect is
    # (sum_k A)=1 per head -> + sum_h b_V[h] @ W_O[h].
    out += np.asarray(b_O, dtype=np.float32)
    out += np.einsum("he,hed->d", np.asarray(b_V, np.float32), np.asarray(W_O, np.float32))
    return out
